# revision 25
# baseline (speedup 1.0000x reference)
"""Trainium2 Bass kernel for nn_BertDeAttention (dual cross-attention BERT block).

Strategy: data-parallel over batch (8 batches -> 8 NeuronCores). Each core runs
both attention branches for its batch:
  c_out = LN(attn(q=qin, kv=cin; Wq,Wk,Wv) @ Wo.T + bo + cin)
  q_out = LN(attn(q=cin, kv=qin; Wqq,Wqk,Wqv) @ Wo.T + bo + cin)

v6 design (engine-balanced; measured per-instr HW costs drove the layout):
  - all matmuls N=512 with rotating stationaries (~221 ns) or K=64 row-tiled
    score pairs (~325 ns/pair)
  - exp is split into two single-PSUM-bank [128,512] activations (435 ns each
    vs 1190 ns for a bank-crossing [128,1024])
  - every PSUM->SBUF move runs on the ACT engine (Copy 491 ns / Identity+bias
    1018 ns per [128,512]) instead of the DVE (1.3 us) -- exp/identity/copy/
    square share one ACT table set, so no table reloads
  - V bias is folded into the residual on the host (ctx' = ctx + bv exactly,
    since softmax rows sum to 1 => bv @ Wo.T joins cin + bo)
  - V tiles are packed [v_h0 | ones | v_h1] per head pair so PV matmuls yield
    context rows and softmax denominators in one pass
  - LayerNorm: y = psum + residual (DVE), bn_stats/bn_aggr (DVE), rsqrt via
    DVE-only Newton iteration, apply via ACT Identity(scale, bias) per bank
"""
import sys
import numpy as np

sys.path.insert(0, "/opt/trn_rl_repo")

import ml_dtypes  # noqa: E402

VERSION = "v6.11"
B, L, HID, NH = 8, 1024, 1024, 16
DH = HID // NH  # 64
NP = 128        # partitions
NCH = HID // NP  # 8 chunks of 128 along any 1024 dim
NPAIR = NH // 2  # 8 head pairs
EPS = 1e-12

_COMPILED = {}


def _build(flags):
    import concourse.bass as bass  # noqa: F401
    import concourse.tile as tile
    from concourse import bacc, mybir

    BF16 = mybir.dt.bfloat16
    FP8 = mybir.dt.float8e4
    F32 = mybir.dt.float32
    Alu = mybir.AluOpType
    Act = mybir.ActivationFunctionType

    has_gb = flags["has_gb"]
    has_qkb = flags["has_qkb"]
    reps = flags.get("reps", 1)
    no_exp = flags.get("no_exp", False)      # diagnostic: fake es tiles
    no_norm = flags.get("no_norm", False)    # diagnostic: skip softmax norm
    no_ln = flags.get("no_ln", False)        # diagnostic: skip LN epilogue
    no_attn = flags.get("no_attn", False)    # diagnostic: fake ctx tiles

    nc = bacc.Bacc("TRN2", target_bir_lowering=False, debug=False)

    # ---- DRAM parameters -------------------------------------------------
    def din(name, shape, dt):
        return nc.dram_tensor(name, shape, dt, kind="ExternalInput").ap()

    xt_c = din("xt_c", [HID, L], BF16)      # cin^T feature-major
    xt_q = din("xt_q", [HID, L], BF16)      # qin^T feature-major
    res_c = din("res_c", [L, HID], F32)     # residual for c branch (token-major)
    res_q = din("res_q", [L, HID], F32)     # residual for q branch
    mask_c = din("mask_c", [NP, NCH], F32)  # mask[k] at [k%128, k//128]
    mask_q = din("mask_q", [NP, NCH], F32)
    wts = {n: din(f"wt_{n}", [HID, HID], BF16)
           for n in ["q", "k", "v", "qq", "qk", "qv", "o"]}  # W.T ([e, o])
    if has_qkb:
        biases = {n: din(f"b_{n}", [NP, NCH], F32)
                  for n in ["q", "k", "qq", "qk"]}           # [o%128, o//128]
    if has_gb:
        gb_in = din("gammabeta", [2, HID], F32)

    c_out = nc.dram_tensor("c_out", [L, HID], F32, kind="ExternalOutput").ap()
    q_out = nc.dram_tensor("q_out", [L, HID], F32, kind="ExternalOutput").ap()

    with tile.TileContext(nc) as tc:
        import contextlib
        ctx = contextlib.ExitStack()
        # SBUF pools
        xpool = ctx.enter_context(tc.tile_pool(name="x", bufs=1))
        wtp = ctx.enter_context(tc.tile_pool(name="wt", bufs=7))
        vp = ctx.enter_context(tc.tile_pool(name="vp", bufs=2))
        qkp = ctx.enter_context(tc.tile_pool(name="qkp", bufs=8))
        esp = ctx.enter_context(tc.tile_pool(name="esp", bufs=2))
        ctxp = ctx.enter_context(tc.tile_pool(name="ctxp", bufs=9))
        rbcp = ctx.enter_context(tc.tile_pool(name="rbcp", bufs=1))
        cpp = ctx.enter_context(tc.tile_pool(name="cpp", bufs=2))
        epi = ctx.enter_context(tc.tile_pool(name="epi", bufs=2))
        cinp = ctx.enter_context(tc.tile_pool(name="cinp", bufs=1))
        outp = ctx.enter_context(tc.tile_pool(name="outp", bufs=2))
        smal = ctx.enter_context(tc.tile_pool(name="smal", bufs=4))
        # PSUM pools (8 banks: st 2x2 + pv 2x1 + proj 2x1)
        stp = ctx.enter_context(tc.tile_pool(name="stp", bufs=2, space="PSUM"))
        pvp = ctx.enter_context(tc.tile_pool(name="pvp", bufs=2, space="PSUM"))
        prp = ctx.enter_context(tc.tile_pool(name="prp", bufs=2, space="PSUM"))

        # ---- resident loads ---------------------------------------------
        xc = xpool.tile([NP, NCH, L], BF16, tag="xc")
        nc.sync.dma_start(xc[:], xt_c.rearrange("(c p) l -> p c l", p=NP))
        xq = xpool.tile([NP, NCH, L], BF16, tag="xq")
        nc.sync.dma_start(xq[:], xt_q.rearrange("(c p) l -> p c l", p=NP))

        mset = {}
        for nm, src in [("c", mask_c), ("q", mask_q)]:
            m = smal.tile([NP, NCH], F32, tag=f"mask{nm}")
            nc.sync.dma_start(m[:], src[:])
            mset[nm] = m
        bset = {}
        if has_qkb:
            for nm in ["q", "k", "qq", "qk"]:
                b = smal.tile([NP, NCH], F32, tag=f"b{nm}")
                nc.sync.dma_start(b[:], biases[nm][:])
                bset[nm] = b
        eps_sb = smal.tile([NP, 1], F32, tag="eps")
        nc.vector.memset(eps_sb[:], EPS)
        if has_gb:
            gb_bc = xpool.tile([NP, 2, HID], F32, tag="gb")
            import concourse.bass as _b
            gb_src = _b.AP(tensor=gb_in.tensor, offset=gb_in.offset,
                           ap=[[0, NP]] + list(gb_in.ap))
            nc.gpsimd.dma_start(gb_bc[:], gb_src)

        def load_wt(name):
            """Load W.T as two o-half tiles [128, 8, 512] on the gpsimd
            (SWDGE) ring so weight traffic doesn't queue behind the SP ring;
            halves let downstream matmuls start after 1MB instead of 2MB."""
            halves = []
            src = wts[name].rearrange("(c p) o -> p c o", p=NP)
            for oh in range(2):
                t = wtp.tile([NP, NCH, 512], BF16, tag="wt")
                nc.gpsimd.dma_start(t[:], src[:, :, oh * 512:(oh + 1) * 512])
                halves.append(t)
            return halves

        if no_exp:
            fake_es = []
            for i in range(2):
                fe = esp.tile([NP, 2, 512], BF16, tag="es")
                nc.vector.memset(fe[:], 0.001)
                fake_es.append(fe)

        # ================= phase-interleaved program =====================
        # Emission order matters: tile-pool slots are granted in emission
        # (FIFO) order, so q's projections are emitted BEFORE c's
        # out-projection — their PSUM/weight allocations then interleave
        # with c's attention instead of queueing behind c's epilogue.
        BRANCHES = {
            "c": (xq, xc, "q", "k", "v", "c", res_c, c_out),
            "q": (xc, xq, "qq", "qk", "qv", "q", res_q, q_out),
        }

        def proj_phase(br):
            xsrc_q, xsrc_kv, wn_q, wn_k, wn_v, msk, res_dram, out_dram = \
                BRANCHES[br]
            # ---- V projection: token-major packed [v_h0 | ones | v_h1]
            # vtile[l, kc, pair, 192]; v bias folded into residual on host.
            wv = load_wt(wn_v)
            vtile = vp.tile([NP, NCH, NPAIR, 192], BF16, tag="vt")
            nc.vector.memset(vtile[:, :, :, 64:128], 1.0)
            for oh in range(2):          # o halves of 512
                for lc in range(NCH):    # token chunks
                    ps = prp.tile([NP, 512], F32, tag="pr")
                    for ec in range(NCH):
                        nc.tensor.matmul(
                            ps[:], xsrc_kv[:, ec, lc * NP:(lc + 1) * NP],
                            wv[oh][:, ec, :],
                            start=(ec == 0), stop=(ec == NCH - 1))
                    # one ACT copy moves all 4 pairs' [h0|h1] halves into
                    # the packed layout (ACT has headroom; DVE stays free
                    # for the softmax-normalize bank-release copies)
                    src = ps[:].rearrange("p (f h d) -> p f h d", f=4, h=2)
                    dst = _vsel(vtile, lc, oh)
                    if (oh + lc) % 2:
                        nc.scalar.activation(dst, src, Act.Copy, scale=1.0)
                    else:
                        nc.vector.tensor_copy(dst, src)
            # ---- Q/K projections: feature-major pair tiles [128, L] fp8
            qt, kt = [], []
            for wn, dst_list, xsrc in [
                (wn_q, qt, xsrc_q),
                (wn_k, kt, xsrc_kv),
            ]:
                w = load_wt(wn)
                kbase = dst_list is kt
                for p in range(NPAIR):
                    t = qkp.tile([NP, L], FP8,
                                 tag="qt" if dst_list is qt else "kt")
                    for lh in range(2):
                        ps = prp.tile([NP, 512], F32, tag="pr")
                        for ec in range(NCH):
                            nc.tensor.matmul(
                                ps[:], w[p // 4][:, ec, (p % 4) * NP:(p % 4 + 1) * NP],
                                xsrc[:, ec, lh * 512:(lh + 1) * 512],
                                start=(ec == 0), stop=(ec == NCH - 1))
                        dst = t[:, lh * 512:(lh + 1) * 512]
                        on_act = (lh == 1) ^ kbase  # alternate cast engines
                        with nc.allow_low_precision(reason="fp8 q/k"):
                            if has_qkb:
                                if on_act:
                                    nc.scalar.activation(
                                        dst, ps[:], Act.Identity,
                                        bias=bset[wn][:, p:p + 1], scale=1.0)
                                else:
                                    nc.vector.tensor_scalar(
                                        out=dst, in0=ps[:],
                                        scalar1=bset[wn][:, p:p + 1],
                                        scalar2=None, op0=Alu.add)
                            elif on_act:
                                nc.scalar.activation(dst, ps[:], Act.Copy,
                                                     scale=1.0)
                            else:
                                nc.vector.tensor_copy(dst, ps[:])
                    dst_list.append(t)
            return vtile, qt, kt

        def attn_phase(br, vtile, qt, kt):
            msk = BRANCHES[br][5]
            ctx_tiles = []
            for p in range(NPAIR):
                cx = ctxp.tile([NP, L], BF16, tag="ctx")
                if no_attn:
                    nc.vector.memset(cx[:], 0.01)
                    ctx_tiles.append(cx)
                    continue
                for qh in range(2):
                    qsl = slice(qh * 512, (qh + 1) * 512)
                    pv0 = pvp.tile([NP, 512], F32, tag="pv")
                    pv1 = pvp.tile([NP, 512], F32, tag="pv")
                    prio = tc.high_priority()
                    prio.__enter__()
                    for kc in range(NCH):
                        st = stp.tile([NP, 1024], F32, tag="st")
                        # packed score matmuls: h0 cols 0:512, h1 512:1024
                        nc.tensor.matmul(
                            st[:, 0:512],
                            kt[p][0:64, kc * NP:(kc + 1) * NP],
                            qt[p][0:64, qsl],
                            start=True, stop=True)
                        nc.tensor.matmul(
                            st[:, 512:1024],
                            kt[p][64:128, kc * NP:(kc + 1) * NP],
                            qt[p][64:128, qsl],
                            start=True, stop=True)
                        if no_exp:
                            es = fake_es[kc % 2]
                        else:
                            es = esp.tile([NP, 2, 512], BF16, tag="es")
                            nc.scalar.activation(
                                es[:].rearrange("p a b -> p (a b)"), st[:],
                                Act.Exp, bias=mset[msk][:, kc:kc + 1],
                                scale=0.125)
                        nc.tensor.matmul(
                            pv0[:], vtile[:, kc, p, 0:128], es[:, 0, :],
                            start=(kc == 0), stop=(kc == NCH - 1))
                        nc.tensor.matmul(
                            pv1[:], vtile[:, kc, p, 64:192], es[:, 1, :],
                            start=(kc == 0), stop=(kc == NCH - 1))
                    prio.__exit__(None, None, None)
                    # softmax-normalize into feature-major ctx pair tile
                    # pv0: rows 0:64 = ctx_h0, 64:128 = rowsum (bcast)
                    # pv1: rows 0:64 = rowsum (bcast), 64:128 = ctx_h1
                    if no_norm:
                        nc.vector.tensor_copy(cx[:, qsl], pv0[:])
                    else:
                        # fast bf16 copies release the two PSUM banks in
                        # ~2.6us so the next pair's PV chains never stall;
                        # the normalize then runs all-bf16 (2x DVE mode)
                        # entirely in SBUF, off the critical path.
                        cpv = cpp.tile([NP, 2, 512], BF16, tag="cpv")
                        # per-bank copies release each PV PSUM bank as soon
                        # as possible -- high priority so queued DVE cast
                        # work never delays the next pair's PV chains
                        with tc.high_priority():
                            nc.vector.tensor_copy(cpv[:, 0, :], pv0[:])
                            nc.vector.tensor_copy(cpv[:, 1, :], pv1[:])
                        rbc = rbcp.tile([NP, 512], BF16, tag="rbc")
                        with nc.allow_low_precision(reason="softmax bf16"):
                            nc.vector.reciprocal(rbc[0:64, :],
                                                 cpv[64:128, 0, :])
                            nc.vector.reciprocal(rbc[64:128, :],
                                                 cpv[0:64, 1, :])
                            nc.vector.tensor_tensor(
                                out=cx[0:64, qsl], in0=cpv[0:64, 0, :],
                                in1=rbc[0:64, :], op=Alu.mult)
                            nc.vector.tensor_tensor(
                                out=cx[64:128, qsl], in0=cpv[64:128, 1, :],
                                in1=rbc[64:128, :], op=Alu.mult)
                ctx_tiles.append(cx)
            return ctx_tiles

        def outproj_phase(br, ctx_tiles):
            res_dram, out_dram = BRANCHES[br][6], BRANCHES[br][7]
            wo = load_wt("o")
            for lc in range(NCH):
                cint = cinp.tile([NP, HID], F32, tag="cin")
                nc.gpsimd.dma_start(cint[:], res_dram[lc * NP:(lc + 1) * NP, :])
                y = epi.tile([NP, HID], F32, tag="y")
                for oh in range(2):
                    ps = prp.tile([NP, 512], F32, tag="pr")
                    for ec in range(NCH):
                        nc.tensor.matmul(
                            ps[:], ctx_tiles[ec][:, lc * NP:(lc + 1) * NP],
                            wo[oh][:, ec, :],
                            start=(ec == 0), stop=(ec == NCH - 1))
                    # residual add: reads PSUM, writes SBUF, frees the bank
                    nc.vector.tensor_tensor(
                        out=y[:, oh * 512:(oh + 1) * 512], in0=ps[:],
                        in1=cint[:, oh * 512:(oh + 1) * 512], op=Alu.add)
                if no_ln:
                    nc.gpsimd.dma_start(out_dram[lc * NP:(lc + 1) * NP, :], y[:])
                    continue
                stats = smal.tile([NP, 2, 6], F32, tag="stats")
                for oh in range(2):
                    nc.vector.bn_stats(stats[:, oh, :],
                                       y[:, oh * 512:(oh + 1) * 512])
                mv = smal.tile([NP, 2], F32, tag="mv")
                nc.vector.bn_aggr(mv[:], stats[:])
                # rstd = rsqrt(var + eps) via DVE-only Newton iteration
                # (keeps the ACT exp table resident: Sqrt lives in another
                # table set and would force a reload amid the attention exps).
                # x0 = min(1, 1/v) converges for any v > 0; var here is ~1.
                w = smal.tile([NP, 3], F32, tag="nwt")
                v_ = w[:, 0:1]
                x_ = w[:, 1:2]
                u_ = w[:, 2:3]
                nc.vector.tensor_scalar(out=v_, in0=mv[:, 1:2],
                                        scalar1=eps_sb[:],
                                        scalar2=None, op0=Alu.add)
                nc.vector.reciprocal(x_, v_)
                nc.vector.tensor_scalar(out=x_, in0=x_, scalar1=1.0,
                                        scalar2=None, op0=Alu.min)
                for _ in range(2):
                    nc.vector.tensor_tensor(out=u_, in0=x_, in1=x_,
                                            op=Alu.mult)
                    nc.vector.tensor_tensor(out=u_, in0=u_, in1=v_,
                                            op=Alu.mult)
                    nc.vector.tensor_scalar(out=u_, in0=u_, scalar1=-0.5,
                                            scalar2=1.5, op0=Alu.mult,
                                            op1=Alu.add)
                    nc.vector.tensor_tensor(out=x_, in0=x_, in1=u_,
                                            op=Alu.mult)
                o = outp.tile([NP, HID], F32, tag="o")
                nc.vector.tensor_scalar(
                    out=o[:], in0=y[:], scalar1=mv[:, 0:1],
                    scalar2=x_, op0=Alu.subtract, op1=Alu.mult)
                if has_gb:
                    nc.vector.tensor_tensor(
                        out=o[:], in0=o[:], in1=gb_bc[:, 0, :], op=Alu.mult)
                    nc.vector.tensor_tensor(
                        out=o[:], in0=o[:], in1=gb_bc[:, 1, :], op=Alu.add)
                nc.gpsimd.dma_start(out_dram[lc * NP:(lc + 1) * NP, :], o[:])

        loop_cm = tc.For_i(0, reps, 1) if reps > 1 else contextlib.nullcontext()
        ctx.enter_context(loop_cm)
        sc = proj_phase("c")
        ctx_c = attn_phase("c", *sc)
        sq = proj_phase("q")          # emitted before c's outproj: overlaps
        outproj_phase("c", ctx_c)     # c's attention on the device
        ctx_q = attn_phase("q", *sq)
        outproj_phase("q", ctx_q)
        ctx.close()
    nc.compile()
    return nc


def _vsel(vtile, lc, oh):
    """AP over vtile[:, lc, oh*4:(oh+1)*4, {0:64 | 128:192}]: the 4 pairs'
    v_h0/v_h1 blocks of one projection half, skipping the ones blocks."""
    import concourse.bass as bass
    base = vtile[:, lc, oh * 4:(oh + 1) * 4, :]  # [p, 4, 192]
    ap = base.ap  # [[pstep,128],[192,4],[1,192]]
    return bass.AP(tensor=base.tensor, offset=base.offset,
                   ap=[ap[0], ap[1], [128, 2], [1, 64]])


def _prep(inputs):
    bf = ml_dtypes.bfloat16

    def t_bf(a):
        return np.ascontiguousarray(np.asarray(a, np.float32).T).astype(bf)

    wts = {}
    for n, key in [("q", "Wq"), ("k", "Wk"), ("v", "Wv"), ("qq", "Wqq"),
                   ("qk", "Wqk"), ("qv", "Wqv"), ("o", "Wo")]:
        wts[n] = t_bf(inputs[key])

    def b_rs(b):
        return np.ascontiguousarray(
            np.asarray(b, np.float32).reshape(NCH, NP).T)

    shared = {f"wt_{n}": w for n, w in wts.items()}
    qkb = [np.asarray(inputs[k], np.float32) for k in ["bq", "bk", "bqq", "bqk"]]
    has_qkb = any(np.any(b != 0.0) for b in qkb)
    if has_qkb:
        for n, b in zip(["q", "k", "qq", "qk"], qkb):
            shared[f"b_{n}"] = b_rs(b)
    gamma = np.asarray(inputs["gamma"], np.float32)
    beta = np.asarray(inputs["beta"], np.float32)
    has_gb = not (np.all(gamma == 1.0) and np.all(beta == 0.0))
    if has_gb:
        shared["gammabeta"] = np.ascontiguousarray(
            np.stack([gamma, beta], 0))

    cin = np.asarray(inputs["cinput_tensor"], np.float32)
    qin = np.asarray(inputs["qinput_tensor"], np.float32)
    Wo = np.asarray(inputs["Wo"], np.float32)
    bo = np.asarray(inputs["bo"], np.float32)
    bv = np.asarray(inputs["bv"], np.float32)
    bqv = np.asarray(inputs["bqv"], np.float32)
    # v-bias folds into the residual exactly: softmax rows sum to 1, so
    # ctx' = ctx + bv and (ctx + bv) @ Wo.T + bo + cin = ctx @ Wo.T + res.
    res_c_extra = bo + bv @ Wo.T
    res_q_extra = bo + bqv @ Wo.T
    am = np.asarray(inputs["attention_mask"], np.float32).reshape(B, L)
    qam = np.asarray(inputs["qattention_mask"], np.float32).reshape(B, L)

    in_maps = []
    for b in range(B):
        m = dict(shared)
        m["xt_c"] = t_bf(cin[b])
        m["xt_q"] = t_bf(qin[b])
        m["res_c"] = np.ascontiguousarray(cin[b] + res_c_extra)
        m["res_q"] = np.ascontiguousarray(cin[b] + res_q_extra)
        m["mask_c"] = np.ascontiguousarray(am[b].reshape(NCH, NP).T)
        m["mask_q"] = np.ascontiguousarray(qam[b].reshape(NCH, NP).T)
        in_maps.append(m)
    return in_maps, has_gb, has_qkb


def kernel(**inputs):
    from concourse.bass_utils import run_bass_kernel_spmd

    in_maps, has_gb, has_qkb = _prep(inputs)
    key = (VERSION, has_gb, has_qkb)
    if key not in _COMPILED:
        _COMPILED[key] = _build({"has_gb": has_gb, "has_qkb": has_qkb})
    nc = _COMPILED[key]
    res = run_bass_kernel_spmd(nc, in_maps, list(range(B)))
    c = np.stack([res.results[b]["c_out"] for b in range(B)], 0)
    q = np.stack([res.results[b]["q_out"] for b in range(B)], 0)
    return (c, q)


# revision 26
# speedup vs baseline: 1.0828x; 1.0828x over previous
"""Trainium2 Bass kernel for nn_BertDeAttention (dual cross-attention BERT block).

Strategy: data-parallel over batch (8 batches -> 8 NeuronCores). Each core runs
both attention branches for its batch:
  c_out = LN(attn(q=qin, kv=cin; Wq,Wk,Wv) @ Wo.T + bo + cin)
  q_out = LN(attn(q=cin, kv=qin; Wqq,Wqk,Wqv) @ Wo.T + bo + cin)

v6 design (engine-balanced; measured per-instr HW costs drove the layout):
  - all matmuls N=512 with rotating stationaries (~221 ns) or K=64 row-tiled
    score pairs (~325 ns/pair)
  - exp is split into two single-PSUM-bank [128,512] activations (435 ns each
    vs 1190 ns for a bank-crossing [128,1024])
  - every PSUM->SBUF move runs on the ACT engine (Copy 491 ns / Identity+bias
    1018 ns per [128,512]) instead of the DVE (1.3 us) -- exp/identity/copy/
    square share one ACT table set, so no table reloads
  - V bias is folded into the residual on the host (ctx' = ctx + bv exactly,
    since softmax rows sum to 1 => bv @ Wo.T joins cin + bo)
  - V tiles are packed [v_h0 | ones | v_h1] per head pair so PV matmuls yield
    context rows and softmax denominators in one pass
  - LayerNorm: y = psum + residual (DVE), bn_stats/bn_aggr (DVE), rsqrt via
    DVE-only Newton iteration, apply via ACT Identity(scale, bias) per bank
"""
import sys
import numpy as np

sys.path.insert(0, "/opt/trn_rl_repo")

import ml_dtypes  # noqa: E402

VERSION = "v6.10"
B, L, HID, NH = 8, 1024, 1024, 16
DH = HID // NH  # 64
NP = 128        # partitions
NCH = HID // NP  # 8 chunks of 128 along any 1024 dim
NPAIR = NH // 2  # 8 head pairs
EPS = 1e-12

_COMPILED = {}


def _build(flags):
    import concourse.bass as bass  # noqa: F401
    import concourse.tile as tile
    from concourse import bacc, mybir

    BF16 = mybir.dt.bfloat16
    FP8 = mybir.dt.float8e4
    F32 = mybir.dt.float32
    Alu = mybir.AluOpType
    Act = mybir.ActivationFunctionType

    has_gb = flags["has_gb"]
    has_qkb = flags["has_qkb"]
    reps = flags.get("reps", 1)
    no_exp = flags.get("no_exp", False)      # diagnostic: fake es tiles
    no_norm = flags.get("no_norm", False)    # diagnostic: skip softmax norm
    no_ln = flags.get("no_ln", False)        # diagnostic: skip LN epilogue
    no_attn = flags.get("no_attn", False)    # diagnostic: fake ctx tiles

    nc = bacc.Bacc("TRN2", target_bir_lowering=False, debug=False)

    # ---- DRAM parameters -------------------------------------------------
    def din(name, shape, dt):
        return nc.dram_tensor(name, shape, dt, kind="ExternalInput").ap()

    xt_c = din("xt_c", [HID, L], BF16)      # cin^T feature-major
    xt_q = din("xt_q", [HID, L], BF16)      # qin^T feature-major
    res_c = din("res_c", [L, HID], F32)     # residual for c branch (token-major)
    res_q = din("res_q", [L, HID], F32)     # residual for q branch
    mask_c = din("mask_c", [NP, NCH], F32)  # mask[k] at [k%128, k//128]
    mask_q = din("mask_q", [NP, NCH], F32)
    wts = {n: din(f"wt_{n}", [HID, HID], BF16)
           for n in ["q", "k", "v", "qq", "qk", "qv", "o"]}  # W.T ([e, o])
    if has_qkb:
        biases = {n: din(f"b_{n}", [NP, NCH], F32)
                  for n in ["q", "k", "qq", "qk"]}           # [o%128, o//128]
    if has_gb:
        gb_in = din("gammabeta", [2, HID], F32)

    c_out = nc.dram_tensor("c_out", [L, HID], F32, kind="ExternalOutput").ap()
    q_out = nc.dram_tensor("q_out", [L, HID], F32, kind="ExternalOutput").ap()

    with tile.TileContext(nc) as tc:
        import contextlib
        ctx = contextlib.ExitStack()
        # SBUF pools
        xpool = ctx.enter_context(tc.tile_pool(name="x", bufs=1))
        wtp = ctx.enter_context(tc.tile_pool(name="wt", bufs=7))
        vp = ctx.enter_context(tc.tile_pool(name="vp", bufs=2))
        qkp = ctx.enter_context(tc.tile_pool(name="qkp", bufs=8))
        esp = ctx.enter_context(tc.tile_pool(name="esp", bufs=2))
        ctxp = ctx.enter_context(tc.tile_pool(name="ctxp", bufs=9))
        rbcp = ctx.enter_context(tc.tile_pool(name="rbcp", bufs=1))
        cpp = ctx.enter_context(tc.tile_pool(name="cpp", bufs=2))
        epi = ctx.enter_context(tc.tile_pool(name="epi", bufs=2))
        cinp = ctx.enter_context(tc.tile_pool(name="cinp", bufs=1))
        outp = ctx.enter_context(tc.tile_pool(name="outp", bufs=2))
        smal = ctx.enter_context(tc.tile_pool(name="smal", bufs=4))
        # PSUM pools (8 banks: st 2x2 + pv 2x1 + proj 2x1)
        stp = ctx.enter_context(tc.tile_pool(name="stp", bufs=2, space="PSUM"))
        pvp = ctx.enter_context(tc.tile_pool(name="pvp", bufs=2, space="PSUM"))
        prp = ctx.enter_context(tc.tile_pool(name="prp", bufs=2, space="PSUM"))

        # ---- resident loads ---------------------------------------------
        xc = xpool.tile([NP, NCH, L], BF16, tag="xc")
        nc.sync.dma_start(xc[:], xt_c.rearrange("(c p) l -> p c l", p=NP))
        xq = xpool.tile([NP, NCH, L], BF16, tag="xq")
        nc.sync.dma_start(xq[:], xt_q.rearrange("(c p) l -> p c l", p=NP))

        mset = {}
        for nm, src in [("c", mask_c), ("q", mask_q)]:
            m = smal.tile([NP, NCH], F32, tag=f"mask{nm}")
            nc.sync.dma_start(m[:], src[:])
            mset[nm] = m
        bset = {}
        if has_qkb:
            for nm in ["q", "k", "qq", "qk"]:
                b = smal.tile([NP, NCH], F32, tag=f"b{nm}")
                nc.sync.dma_start(b[:], biases[nm][:])
                bset[nm] = b
        eps_sb = smal.tile([NP, 1], F32, tag="eps")
        nc.vector.memset(eps_sb[:], EPS)
        if has_gb:
            gb_bc = xpool.tile([NP, 2, HID], F32, tag="gb")
            import concourse.bass as _b
            gb_src = _b.AP(tensor=gb_in.tensor, offset=gb_in.offset,
                           ap=[[0, NP]] + list(gb_in.ap))
            nc.gpsimd.dma_start(gb_bc[:], gb_src)

        def load_wt(name):
            """Load W.T as two o-half tiles [128, 8, 512] on the gpsimd
            (SWDGE) ring so weight traffic doesn't queue behind the SP ring;
            halves let downstream matmuls start after 1MB instead of 2MB."""
            halves = []
            src = wts[name].rearrange("(c p) o -> p c o", p=NP)
            for oh in range(2):
                t = wtp.tile([NP, NCH, 512], BF16, tag="wt")
                nc.gpsimd.dma_start(t[:], src[:, :, oh * 512:(oh + 1) * 512])
                halves.append(t)
            return halves

        if no_exp:
            fake_es = []
            for i in range(2):
                fe = esp.tile([NP, 2, 512], BF16, tag="es")
                nc.vector.memset(fe[:], 0.001)
                fake_es.append(fe)

        # ================= phase-interleaved program =====================
        # Emission order matters: tile-pool slots are granted in emission
        # (FIFO) order, so q's projections are emitted BEFORE c's
        # out-projection — their PSUM/weight allocations then interleave
        # with c's attention instead of queueing behind c's epilogue.
        BRANCHES = {
            "c": (xq, xc, "q", "k", "v", "c", res_c, c_out),
            "q": (xc, xq, "qq", "qk", "qv", "q", res_q, q_out),
        }

        def proj_phase(br):
            xsrc_q, xsrc_kv, wn_q, wn_k, wn_v, msk, res_dram, out_dram = \
                BRANCHES[br]
            # ---- V projection: token-major packed [v_h0 | ones | v_h1]
            # vtile[l, kc, pair, 192]; v bias folded into residual on host.
            wv = load_wt(wn_v)
            vtile = vp.tile([NP, NCH, NPAIR, 192], BF16, tag="vt")
            nc.vector.memset(vtile[:, :, :, 64:128], 1.0)
            for oh in range(2):          # o halves of 512
                for lc in range(NCH):    # token chunks
                    ps = prp.tile([NP, 512], F32, tag="pr")
                    for ec in range(NCH):
                        nc.tensor.matmul(
                            ps[:], xsrc_kv[:, ec, lc * NP:(lc + 1) * NP],
                            wv[oh][:, ec, :],
                            start=(ec == 0), stop=(ec == NCH - 1))
                    # one ACT copy moves all 4 pairs' [h0|h1] halves into
                    # the packed layout (ACT has headroom; DVE stays free
                    # for the softmax-normalize bank-release copies)
                    src = ps[:].rearrange("p (f h d) -> p f h d", f=4, h=2)
                    dst = _vsel(vtile, lc, oh)
                    if (oh + lc) % 2:
                        nc.scalar.activation(dst, src, Act.Copy, scale=1.0)
                    else:
                        nc.vector.tensor_copy(dst, src)
            # ---- Q/K projections: feature-major pair tiles [128, L] fp8
            qt, kt = [], []
            for wn, dst_list, xsrc in [
                (wn_q, qt, xsrc_q),
                (wn_k, kt, xsrc_kv),
            ]:
                w = load_wt(wn)
                kbase = dst_list is kt
                for p in range(NPAIR):
                    t = qkp.tile([NP, L], FP8,
                                 tag="qt" if dst_list is qt else "kt")
                    for lh in range(2):
                        ps = prp.tile([NP, 512], F32, tag="pr")
                        for ec in range(NCH):
                            nc.tensor.matmul(
                                ps[:], w[p // 4][:, ec, (p % 4) * NP:(p % 4 + 1) * NP],
                                xsrc[:, ec, lh * 512:(lh + 1) * 512],
                                start=(ec == 0), stop=(ec == NCH - 1))
                        dst = t[:, lh * 512:(lh + 1) * 512]
                        on_act = (lh == 1) ^ kbase  # alternate cast engines
                        with nc.allow_low_precision(reason="fp8 q/k"):
                            if has_qkb:
                                if on_act:
                                    nc.scalar.activation(
                                        dst, ps[:], Act.Identity,
                                        bias=bset[wn][:, p:p + 1], scale=1.0)
                                else:
                                    nc.vector.tensor_scalar(
                                        out=dst, in0=ps[:],
                                        scalar1=bset[wn][:, p:p + 1],
                                        scalar2=None, op0=Alu.add)
                            elif on_act:
                                nc.scalar.activation(dst, ps[:], Act.Copy,
                                                     scale=1.0)
                            else:
                                nc.vector.tensor_copy(dst, ps[:])
                    dst_list.append(t)
            return vtile, qt, kt

        def attn_phase(br, vtile, qt, kt):
            msk = BRANCHES[br][5]
            ctx_tiles = []
            for p in range(NPAIR):
                cx = ctxp.tile([NP, L], BF16, tag="ctx")
                if no_attn:
                    nc.vector.memset(cx[:], 0.01)
                    ctx_tiles.append(cx)
                    continue
                for qh in range(2):
                    qsl = slice(qh * 512, (qh + 1) * 512)
                    pv0 = pvp.tile([NP, 512], F32, tag="pv")
                    pv1 = pvp.tile([NP, 512], F32, tag="pv")
                    for kc in range(NCH):
                        st = stp.tile([NP, 1024], F32, tag="st")
                        # packed score matmuls: h0 cols 0:512, h1 512:1024
                        nc.tensor.matmul(
                            st[:, 0:512],
                            kt[p][0:64, kc * NP:(kc + 1) * NP],
                            qt[p][0:64, qsl],
                            start=True, stop=True)
                        nc.tensor.matmul(
                            st[:, 512:1024],
                            kt[p][64:128, kc * NP:(kc + 1) * NP],
                            qt[p][64:128, qsl],
                            start=True, stop=True)
                        if no_exp:
                            es = fake_es[kc % 2]
                        else:
                            es = esp.tile([NP, 2, 512], BF16, tag="es")
                            nc.scalar.activation(
                                es[:].rearrange("p a b -> p (a b)"), st[:],
                                Act.Exp, bias=mset[msk][:, kc:kc + 1],
                                scale=0.125)
                        nc.tensor.matmul(
                            pv0[:], vtile[:, kc, p, 0:128], es[:, 0, :],
                            start=(kc == 0), stop=(kc == NCH - 1))
                        nc.tensor.matmul(
                            pv1[:], vtile[:, kc, p, 64:192], es[:, 1, :],
                            start=(kc == 0), stop=(kc == NCH - 1))
                    # softmax-normalize into feature-major ctx pair tile
                    # pv0: rows 0:64 = ctx_h0, 64:128 = rowsum (bcast)
                    # pv1: rows 0:64 = rowsum (bcast), 64:128 = ctx_h1
                    if no_norm:
                        nc.vector.tensor_copy(cx[:, qsl], pv0[:])
                    else:
                        # fast bf16 copies release the two PSUM banks in
                        # ~2.6us so the next pair's PV chains never stall;
                        # the normalize then runs all-bf16 (2x DVE mode)
                        # entirely in SBUF, off the critical path.
                        cpv = cpp.tile([NP, 2, 512], BF16, tag="cpv")
                        # per-bank copies release each PV PSUM bank as soon
                        # as possible -- high priority so queued DVE cast
                        # work never delays the next pair's PV chains
                        with tc.high_priority():
                            nc.vector.tensor_copy(cpv[:, 0, :], pv0[:])
                            nc.vector.tensor_copy(cpv[:, 1, :], pv1[:])
                        rbc = rbcp.tile([NP, 512], BF16, tag="rbc")
                        with nc.allow_low_precision(reason="softmax bf16"):
                            nc.vector.reciprocal(rbc[0:64, :],
                                                 cpv[64:128, 0, :])
                            nc.vector.reciprocal(rbc[64:128, :],
                                                 cpv[0:64, 1, :])
                            nc.vector.tensor_tensor(
                                out=cx[0:64, qsl], in0=cpv[0:64, 0, :],
                                in1=rbc[0:64, :], op=Alu.mult)
                            nc.vector.tensor_tensor(
                                out=cx[64:128, qsl], in0=cpv[64:128, 1, :],
                                in1=rbc[64:128, :], op=Alu.mult)
                ctx_tiles.append(cx)
            return ctx_tiles

        def outproj_phase(br, ctx_tiles):
            res_dram, out_dram = BRANCHES[br][6], BRANCHES[br][7]
            wo = load_wt("o")
            for lc in range(NCH):
                cint = cinp.tile([NP, HID], F32, tag="cin")
                nc.gpsimd.dma_start(cint[:], res_dram[lc * NP:(lc + 1) * NP, :])
                y = epi.tile([NP, HID], F32, tag="y")
                for oh in range(2):
                    ps = prp.tile([NP, 512], F32, tag="pr")
                    for ec in range(NCH):
                        nc.tensor.matmul(
                            ps[:], ctx_tiles[ec][:, lc * NP:(lc + 1) * NP],
                            wo[oh][:, ec, :],
                            start=(ec == 0), stop=(ec == NCH - 1))
                    # residual add: reads PSUM, writes SBUF, frees the bank
                    nc.vector.tensor_tensor(
                        out=y[:, oh * 512:(oh + 1) * 512], in0=ps[:],
                        in1=cint[:, oh * 512:(oh + 1) * 512], op=Alu.add)
                if no_ln:
                    nc.gpsimd.dma_start(out_dram[lc * NP:(lc + 1) * NP, :], y[:])
                    continue
                stats = smal.tile([NP, 2, 6], F32, tag="stats")
                for oh in range(2):
                    nc.vector.bn_stats(stats[:, oh, :],
                                       y[:, oh * 512:(oh + 1) * 512])
                mv = smal.tile([NP, 2], F32, tag="mv")
                nc.vector.bn_aggr(mv[:], stats[:])
                # rstd = rsqrt(var + eps) via DVE-only Newton iteration
                # (keeps the ACT exp table resident: Sqrt lives in another
                # table set and would force a reload amid the attention exps).
                # x0 = min(1, 1/v) converges for any v > 0; var here is ~1.
                w = smal.tile([NP, 3], F32, tag="nwt")
                v_ = w[:, 0:1]
                x_ = w[:, 1:2]
                u_ = w[:, 2:3]
                nc.vector.tensor_scalar(out=v_, in0=mv[:, 1:2],
                                        scalar1=eps_sb[:],
                                        scalar2=None, op0=Alu.add)
                nc.vector.reciprocal(x_, v_)
                nc.vector.tensor_scalar(out=x_, in0=x_, scalar1=1.0,
                                        scalar2=None, op0=Alu.min)
                for _ in range(2):
                    nc.vector.tensor_tensor(out=u_, in0=x_, in1=x_,
                                            op=Alu.mult)
                    nc.vector.tensor_tensor(out=u_, in0=u_, in1=v_,
                                            op=Alu.mult)
                    nc.vector.tensor_scalar(out=u_, in0=u_, scalar1=-0.5,
                                            scalar2=1.5, op0=Alu.mult,
                                            op1=Alu.add)
                    nc.vector.tensor_tensor(out=x_, in0=x_, in1=u_,
                                            op=Alu.mult)
                o = outp.tile([NP, HID], F32, tag="o")
                nc.vector.tensor_scalar(
                    out=o[:], in0=y[:], scalar1=mv[:, 0:1],
                    scalar2=x_, op0=Alu.subtract, op1=Alu.mult)
                if has_gb:
                    nc.vector.tensor_tensor(
                        out=o[:], in0=o[:], in1=gb_bc[:, 0, :], op=Alu.mult)
                    nc.vector.tensor_tensor(
                        out=o[:], in0=o[:], in1=gb_bc[:, 1, :], op=Alu.add)
                nc.gpsimd.dma_start(out_dram[lc * NP:(lc + 1) * NP, :], o[:])

        loop_cm = tc.For_i(0, reps, 1) if reps > 1 else contextlib.nullcontext()
        ctx.enter_context(loop_cm)
        sc = proj_phase("c")
        ctx_c = attn_phase("c", *sc)
        sq = proj_phase("q")          # emitted before c's outproj: overlaps
        outproj_phase("c", ctx_c)     # c's attention on the device
        ctx_q = attn_phase("q", *sq)
        outproj_phase("q", ctx_q)
        ctx.close()
    nc.compile()
    return nc


def _vsel(vtile, lc, oh):
    """AP over vtile[:, lc, oh*4:(oh+1)*4, {0:64 | 128:192}]: the 4 pairs'
    v_h0/v_h1 blocks of one projection half, skipping the ones blocks."""
    import concourse.bass as bass
    base = vtile[:, lc, oh * 4:(oh + 1) * 4, :]  # [p, 4, 192]
    ap = base.ap  # [[pstep,128],[192,4],[1,192]]
    return bass.AP(tensor=base.tensor, offset=base.offset,
                   ap=[ap[0], ap[1], [128, 2], [1, 64]])


def _prep(inputs):
    bf = ml_dtypes.bfloat16

    def t_bf(a):
        return np.ascontiguousarray(np.asarray(a, np.float32).T).astype(bf)

    wts = {}
    for n, key in [("q", "Wq"), ("k", "Wk"), ("v", "Wv"), ("qq", "Wqq"),
                   ("qk", "Wqk"), ("qv", "Wqv"), ("o", "Wo")]:
        wts[n] = t_bf(inputs[key])

    def b_rs(b):
        return np.ascontiguousarray(
            np.asarray(b, np.float32).reshape(NCH, NP).T)

    shared = {f"wt_{n}": w for n, w in wts.items()}
    qkb = [np.asarray(inputs[k], np.float32) for k in ["bq", "bk", "bqq", "bqk"]]
    has_qkb = any(np.any(b != 0.0) for b in qkb)
    if has_qkb:
        for n, b in zip(["q", "k", "qq", "qk"], qkb):
            shared[f"b_{n}"] = b_rs(b)
    gamma = np.asarray(inputs["gamma"], np.float32)
    beta = np.asarray(inputs["beta"], np.float32)
    has_gb = not (np.all(gamma == 1.0) and np.all(beta == 0.0))
    if has_gb:
        shared["gammabeta"] = np.ascontiguousarray(
            np.stack([gamma, beta], 0))

    cin = np.asarray(inputs["cinput_tensor"], np.float32)
    qin = np.asarray(inputs["qinput_tensor"], np.float32)
    Wo = np.asarray(inputs["Wo"], np.float32)
    bo = np.asarray(inputs["bo"], np.float32)
    bv = np.asarray(inputs["bv"], np.float32)
    bqv = np.asarray(inputs["bqv"], np.float32)
    # v-bias folds into the residual exactly: softmax rows sum to 1, so
    # ctx' = ctx + bv and (ctx + bv) @ Wo.T + bo + cin = ctx @ Wo.T + res.
    res_c_extra = bo + bv @ Wo.T
    res_q_extra = bo + bqv @ Wo.T
    am = np.asarray(inputs["attention_mask"], np.float32).reshape(B, L)
    qam = np.asarray(inputs["qattention_mask"], np.float32).reshape(B, L)

    in_maps = []
    for b in range(B):
        m = dict(shared)
        m["xt_c"] = t_bf(cin[b])
        m["xt_q"] = t_bf(qin[b])
        m["res_c"] = np.ascontiguousarray(cin[b] + res_c_extra)
        m["res_q"] = np.ascontiguousarray(cin[b] + res_q_extra)
        m["mask_c"] = np.ascontiguousarray(am[b].reshape(NCH, NP).T)
        m["mask_q"] = np.ascontiguousarray(qam[b].reshape(NCH, NP).T)
        in_maps.append(m)
    return in_maps, has_gb, has_qkb


def kernel(**inputs):
    from concourse.bass_utils import run_bass_kernel_spmd

    in_maps, has_gb, has_qkb = _prep(inputs)
    key = (VERSION, has_gb, has_qkb)
    if key not in _COMPILED:
        _COMPILED[key] = _build({"has_gb": has_gb, "has_qkb": has_qkb})
    nc = _COMPILED[key]
    res = run_bass_kernel_spmd(nc, in_maps, list(range(B)))
    c = np.stack([res.results[b]["c_out"] for b in range(B)], 0)
    q = np.stack([res.results[b]["q_out"] for b in range(B)], 0)
    return (c, q)
